# revision 1
# baseline (speedup 1.0000x reference)
"""Autoregressive LSTM decoder (B=128, S=128, F=512, H=1024) on 8 TRN2 cores.

Strategy: data-parallel over batch (16 samples/core), weights replicated.
All state is kept TRANSPOSED ([dim, batch]) so that
  - gates GEMM: out[gate_dim, batch] = W_tile.T-stationary @ xh-tile streaming,
    with the gate dim on PSUM partitions -> full-width elementwise ops,
  - the fc output y^T is directly the next step's x^T lhsT tiles (zero
    on-device transposes; all layout marshalling happens on the host).
Weights/activations enter the PE in fp16 (fp32 accumulation in PSUM); the
cell state c stays fp32. Measured end-to-end error vs the fp32 reference
is ~5e-4 absmax (the LSTM recurrence damps per-step rounding).
"""
import sys
sys.path.insert(0, "/opt/trn_rl_repo")
import numpy as np

B, S, F, H = 128, 128, 512, 1024
NCORES = 8
BL = B // NCORES          # 16 samples per core
KX = F // 128             # 4  x k-tiles
KH = H // 128             # 8  h k-tiles
KT = KX + KH              # 12 gates k-tiles
MG = 4 * H // 128         # 32 gates m-tiles
MF = F // 128             # 4  fc m-tiles
NSTEPS = S
# Timing aid: run NSTEPS steps but only DMA out the last OUT_STEPS of them,
# so kernels with different NSTEPS have identical I/O footprints.
OUT_STEPS = None  # None -> NSTEPS
# Split each [128K,128M] weight tile into 4 column sub-tiles loaded into
# independent 32-col PE array groups (concurrent LDWEIGHTS streams).
COL_TILING = False

_CACHE = {}


def _split_sync_waits(nc, mybir, limit=1):
    """This toolchain's walrus accepts at most one semaphore wait per
    instruction; move the excess onto preceding same-engine NOPs."""
    cur_insts = nc.cur_bb.bb.instructions
    for f in nc.m.functions:
        for blk in f.blocks:
            insts = blk.instructions
            i = 0
            while i < len(insts):
                inst = insts[i]
                si = inst.sync_info
                if si and si.on_wait and len(si.on_wait) > limit:
                    waits = list(si.on_wait)
                    overflow, keep = waits[:-limit], waits[-limit:]
                    n_nops = 0
                    for j in range(0, len(overflow), limit):
                        chunk = overflow[j:j + limit]
                        nc.engines[inst.engine].nop(nofuse=True)
                        tail = cur_insts.pop()
                        assert "NoOp" in type(tail).__name__, type(tail).__name__
                        tail.sync_info = mybir.SyncInfo(on_wait=list(chunk), on_update=[])
                        insts.insert(i + n_nops, tail)
                        n_nops += 1
                    i += n_nops
                    inst.sync_info = mybir.SyncInfo(on_wait=keep, on_update=list(si.on_update))
                i += 1


def _build():
    if "nc" in _CACHE:
        return _CACHE["nc"]
    import concourse.bass as bass
    import concourse.mybir as mybir
    import concourse.tile as tile

    f16, f32 = mybir.dt.float16, mybir.dt.float32
    nc = bass.Bass()

    wg = nc.dram_tensor("wg", [KT, 128, 4 * H], f16, kind="ExternalInput")
    wf = nc.dram_tensor("wf", [KH, 128, F], f16, kind="ExternalInput")
    bg = nc.dram_tensor("bg", [128, MG * BL], f32, kind="ExternalInput")
    bf = nc.dram_tensor("bf", [128, MF * BL], f32, kind="ExternalInput")
    x0 = nc.dram_tensor("x0", [128, KX * BL], f16, kind="ExternalInput")
    h0 = nc.dram_tensor("h0", [128, KH * BL], f16, kind="ExternalInput")
    c0 = nc.dram_tensor("c0", [128, KH * BL], f32, kind="ExternalInput")
    out_steps = OUT_STEPS or NSTEPS
    yt = nc.dram_tensor("yt", [out_steps, 128, MF * BL], f32, kind="ExternalOutput")

    with tile.TileContext(nc) as tc:
        with (
            tc.tile_pool(name="wpool", bufs=1) as wpool,
            tc.tile_pool(name="state", bufs=3) as st,
            tc.tile_pool(name="work", bufs=2) as wk,
            tc.tile_pool(name="psum", bufs=2, space="PSUM") as pp,
        ):
            wg_sb = []
            for k in range(KT):
                t = wpool.tile([128, 4 * H], f16, tag=f"wg{k}")
                nc.sync.dma_start(t[:], wg[k])
                wg_sb.append(t)
            wf_sb = []
            for k in range(KH):
                t = wpool.tile([128, F], f16, tag=f"wf{k}")
                nc.sync.dma_start(t[:], wf[k])
                wf_sb.append(t)
            bg_sb = wpool.tile([128, MG * BL], f32, tag="bg")
            nc.sync.dma_start(bg_sb[:], bg[:])
            bf_sb = wpool.tile([128, MF * BL], f32, tag="bf")
            nc.sync.dma_start(bf_sb[:], bf[:])

            xT = st.tile([128, KX * BL], f16, tag="xT")
            nc.sync.dma_start(xT[:], x0[:])
            hT = st.tile([128, KH * BL], f16, tag="hT")
            nc.sync.dma_start(hT[:], h0[:])
            cT = st.tile([128, KH * BL], f32, tag="cT")
            nc.sync.dma_start(cT[:], c0[:])

            Sig = mybir.ActivationFunctionType.Sigmoid
            Tanh = mybir.ActivationFunctionType.Tanh
            # k issue order: h-dependent tiles first so the PE works on them
            # while the fc->tanh->x chain of this step boundary completes.
            korder = list(range(KX, KT)) + list(range(KX))

            for t in range(NSTEPS):
                psg = pp.tile([128, MG * BL], f32, tag="psg")
                for ki, k in enumerate(korder):
                    if k < KX:
                        rhs = xT[:, BL * k:BL * (k + 1)]
                    else:
                        rhs = hT[:, BL * (k - KX):BL * (k - KX + 1)]
                    for m in range(MG):
                        # One accumulation group per PSUM bank: start zeroes the
                        # whole 2KB zero-region, so only the first matmul into
                        # the bank may carry start=True.
                        if COL_TILING:
                            for cq in range(4):
                                nc.tensor.matmul(
                                    psg[32 * cq:32 * (cq + 1), BL * m:BL * (m + 1)],
                                    wg_sb[k][:, 128 * m + 32 * cq:128 * m + 32 * (cq + 1)],
                                    rhs,
                                    start=(ki == 0 and m == 0),
                                    stop=(ki == KT - 1 and m == MG - 1),
                                    tile_position=(0, 32 * cq),
                                )
                        else:
                            nc.tensor.matmul(
                                psg[:, BL * m:BL * (m + 1)],
                                wg_sb[k][:, 128 * m:128 * (m + 1)],
                                rhs,
                                start=(ki == 0 and m == 0),
                                stop=(ki == KT - 1 and m == MG - 1),
                            )

                # layout: cols [0:128]=i, [128:256]=f, [256:384]=g, [384:512]=o
                # Bias-add in two halves so the first ACT starts earlier.
                gsb = wk.tile([128, MG * BL], f32, tag="gsb")
                nc.vector.tensor_add(gsb[:, 0:256], psg[:, 0:256], bg_sb[:, 0:256])
                sif = wk.tile([128, 256], f32, tag="sif")
                nc.scalar.activation(sif[:], gsb[:, 0:256], Sig)
                # c2 = sigmoid(f)*c as soon as sif lands
                cT2 = st.tile([128, KH * BL], f32, tag="cT")
                nc.vector.tensor_mul(cT2[:], sif[:, 128:256], cT[:])
                nc.vector.tensor_add(gsb[:, 256:512], psg[:, 256:512], bg_sb[:, 256:512])
                tng = wk.tile([128, 128], f32, tag="tng")
                nc.scalar.activation(tng[:], gsb[:, 256:384], Tanh)
                sgo = wk.tile([128, 128], f32, tag="sgo")
                nc.scalar.activation(sgo[:], gsb[:, 384:512], Sig)

                tmp = wk.tile([128, 128], f32, tag="tmp")
                nc.vector.tensor_mul(tmp[:], sif[:, 0:128], tng[:])
                nc.vector.tensor_add(cT2[:], cT2[:], tmp[:])
                # tanh(c) -> h in half-width chunks: the first 64 cols of hT2
                # land one ACT op earlier, releasing the fc GEMM's first
                # k-tiles sooner (cost-model verified: -119 ns/step).
                tnc = wk.tile([128, 128], f32, tag="tnc")
                hT2 = st.tile([128, KH * BL], f16, tag="hT")
                nc.scalar.activation(tnc[:, 0:64], cT2[:, 0:64], Tanh)
                nc.vector.tensor_mul(hT2[:, 0:64], sgo[:, 0:64], tnc[:, 0:64])
                nc.scalar.activation(tnc[:, 64:128], cT2[:, 64:128], Tanh)
                nc.vector.tensor_mul(hT2[:, 64:128], sgo[:, 64:128], tnc[:, 64:128])

                psy = pp.tile([128, MF * BL], f32, tag="psy")
                for ki in range(KH):
                    for m in range(MF):
                        if COL_TILING:
                            for cq in range(4):
                                nc.tensor.matmul(
                                    psy[32 * cq:32 * (cq + 1), BL * m:BL * (m + 1)],
                                    wf_sb[ki][:, 128 * m + 32 * cq:128 * m + 32 * (cq + 1)],
                                    hT2[:, BL * ki:BL * (ki + 1)],
                                    start=(ki == 0 and m == 0),
                                    stop=(ki == KH - 1 and m == MF - 1),
                                    tile_position=(0, 32 * cq),
                                )
                        else:
                            nc.tensor.matmul(
                                psy[:, BL * m:BL * (m + 1)],
                                wf_sb[ki][:, 128 * m:128 * (m + 1)],
                                hT2[:, BL * ki:BL * (ki + 1)],
                                start=(ki == 0 and m == 0),
                                stop=(ki == KH - 1 and m == MF - 1),
                            )
                ysb = wk.tile([128, MF * BL], f32, tag="ysb")
                nc.vector.tensor_add(ysb[:], psy[:], bf_sb[:])
                yout = wk.tile([128, MF * BL], f32, tag="yout")
                nc.scalar.activation(yout[:], ysb[:], Tanh)
                xT2 = st.tile([128, KX * BL], f16, tag="xT")
                nc.scalar.activation(xT2[:], ysb[:], Tanh)
                if t >= NSTEPS - out_steps:
                    nc.sync.dma_start(yt[t - (NSTEPS - out_steps)], yout[:])

                xT, hT, cT = xT2, hT2, cT2

    _split_sync_waits(nc, mybir, 1)
    _CACHE["nc"] = nc
    return nc


def kernel(input, h0, c0, W_ih, W_hh, b_ih, b_hh, fc_W, fc_b):
    from concourse.bass_utils import run_bass_kernel_spmd

    nc = _build()

    input = np.asarray(input, np.float32)
    h0 = np.asarray(h0, np.float32)
    c0 = np.asarray(c0, np.float32)
    W4 = np.concatenate([np.asarray(W_ih, np.float32),
                         np.asarray(W_hh, np.float32)], axis=1)  # [4H, F+H]
    # wg[k, p, j] = W4[j, 128k+p]
    wg_np = np.ascontiguousarray(
        W4.T.reshape(KT, 128, 4 * H).astype(np.float16))
    wf_np = np.ascontiguousarray(
        np.asarray(fc_W, np.float32).T.reshape(KH, 128, F).astype(np.float16))
    b4 = (np.asarray(b_ih, np.float32) + np.asarray(b_hh, np.float32))
    bg_np = np.ascontiguousarray(
        np.repeat(b4.reshape(MG, 128).T[:, :, None], BL, axis=2).reshape(128, MG * BL)
    ).astype(np.float32)
    bf_np = np.ascontiguousarray(
        np.repeat(np.asarray(fc_b, np.float32).reshape(MF, 128).T[:, :, None],
                  BL, axis=2).reshape(128, MF * BL)).astype(np.float32)

    def tconv(a, kt, dt):
        # a: [BL, kt*128] -> [128, kt*BL] with col 16k+b = a[b, 128k+p]
        return np.ascontiguousarray(
            a.T.reshape(kt, 128, BL).transpose(1, 0, 2).reshape(128, kt * BL)
        ).astype(dt)

    in_maps = []
    for c in range(NCORES):
        b0 = c * BL
        in_maps.append({
            "wg": wg_np, "wf": wf_np, "bg": bg_np, "bf": bf_np,
            "x0": tconv(input[b0:b0 + BL, 0, :], KX, np.float16),
            "h0": tconv(h0[b0:b0 + BL], KH, np.float16),
            "c0": tconv(c0[b0:b0 + BL], KH, np.float32),
        })

    res = run_bass_kernel_spmd(nc, in_maps, list(range(NCORES)))

    out = np.empty((B, S, F), np.float32)
    for c in range(NCORES):
        ytv = res.results[c]["yt"]  # [S, 128, MF*BL]
        # yt[t, p, 16m+b] = y_t[b, 128m+p]
        out[c * BL:(c + 1) * BL] = (
            ytv.reshape(S, 128, MF, BL).transpose(3, 0, 2, 1).reshape(BL, S, F)
        )
    return out

